# revision 49
# baseline (speedup 1.0000x reference)
"""Trainium2 Bass kernel for nn_DQN_57904749085018 (gnn_message_passing).

Computation (reference semantics):
    g   = x[:, idx]                                  [B, S, L] gather
    h   = (g - mean) * rsqrt(var+eps) * gamma + beta [B, S, L] batchnorm (eval)
    h1  = tanh(einsum('bsl,sol->bso', h, W1) + b1)   [B, S, 3]
    h2  = tanh(einsum('bsk,sok->bso', h1, W2) + b2)  [B, S, 2]
    a, sb = h2[..., 0], h2[..., 1]
    out[b,i,j] = tanh(a[b,i]*W3[i,j,0] + sb[b,j]*W3[i,j,1] + b3[i,j])
    -> reshape [B, S*S]

Kernel strategy (pure data parallel over 8 cores, batch-sharded), fp16
end-to-end (measured rel err ~5e-4 against the fp64 oracle, tolerance 2e-2):
  * gather + batchnorm + Linear1 fold into one dense matmul x @ Weff.T;
    x is padded to 512 features with a ones column at 407 so the biases
    ride along as ordinary weight rows.  x is transposed on the host.
  * the pairwise head out[b,(i,j)] = tanh(a_i w0_ij + sb_j w1_ij + b_ij)
    runs as a SINGLE K=128 matmul per output span: the output columns are
    split into four 2700-col windows (27 consecutive i-servers each), and
    per window a combined stationary tile holds sb (rows 0..99), a ones
    row carrying b3 (row 100), and the window's 27 a-rows (rows 101..127).
    The a-rows land at partitions 101..127 directly out of the W2 head by
    computing that head with a column-rearranged stationary into PSUM
    partitions 96..127 (no cross-partition copies exist on any engine).
    The matching table packs W3[:,:,1] (one-hot in j), b3, and the
    windowed W3[:,:,0] rows into one [128, S*S] fp16 operand.
  * the final tanh runs on the scalar engine for output cols 0..8191 and
    as a cubic `z - z^3/3` on the (otherwise idle) vector engine for cols
    8192..9999 (|z| <= 3/sqrt(300) = 0.174 so the cubic errs < 2.2e-5),
    removing the scalar engine as the sole steady-state bottleneck.
  * output is written as fp16 (half the HBM write traffic) in three
    contiguous chunks per 128-row block on the sync ring; the host
    upcasts to fp32.
  * all bulk loads go through gpsimd (SWDGE) as full-128-partition
    transfers: HWDGE-dynamic and partial-partition loads clump onto one
    or two SDMA engines (~27 GiB/s) instead of spreading across all 16.
"""

import sys

import numpy as np

if "/opt/trn_rl_repo" not in sys.path:
    sys.path.insert(0, "/opt/trn_rl_repo")

import concourse.bacc as bacc
import concourse.mybir as mybir
from concourse import bass_utils
from concourse.alu_op_type import AluOpType
from concourse.tile import TileContext

S = 100
L = 13
FEAT = 4 * S + 7  # 407
FP = 512  # padded feature width (col 407 = ones, 408.. = zero)
B = 8192
EPS = 1e-5
N_CORES = 8
BL = B // N_CORES  # 1024 batch rows per core
SS = S * S  # 10000
WSPAN = 2700  # output window: 27 consecutive i-servers
NWIN = 4  # windows 0..2 cover 2700 cols, window 3 covers 1900
F16 = mybir.dt.float16
F32 = mybir.dt.float32

# wt pack layout (columns of the [128, WT_COLS] fp16 tile)
WT_WEFF = [0, 300, 600, 900]  # WeffT feature-chunk k at col 300k, width 300
WT_W2 = [1200, 1400, 1600]  # W2effT k-chunk c, width 200 (cols 100..199 = sb)
WT_W2V = 1800  # 12 blocks of 64: a-head variant for window w, chunk c
WT_COLS = 1800 + NWIN * 3 * 64  # 2568

# final-stage matmul segments: 512-col PSUM-bank grid split at window edges
_EDGES = sorted(set(list(range(0, SS, 512)) + [WSPAN, 2 * WSPAN, 3 * WSPAN, SS]))
SEGS = [(a, b - a) for a, b in zip(_EDGES, _EDGES[1:])]

_module_cache = None


def _build_indices():
    idx = [[2 * i, 2 * i + 1] for i in range(S)]
    start = 2 * S
    for k in range(S):
        u, v = k, (k + 1) % S
        idx[u].extend([start, start + 1])
        idx[v].extend([start, start + 1])
        start += 2
    g0 = 4 * S
    for i in range(S):
        idx[i].extend(range(g0, g0 + 7))
    return np.asarray(idx, dtype=np.int64)


def _host_weights(inputs):
    f64 = np.float64
    gamma = np.asarray(inputs["gamma"], f64)
    beta = np.asarray(inputs["beta"], f64)
    mean = np.asarray(inputs["mean"], f64)
    var = np.asarray(inputs["var"], f64)
    W1 = np.asarray(inputs["W1"], f64)  # [S, 3, L]
    b1 = np.asarray(inputs["b1"], f64)  # [S, 3]
    W2 = np.asarray(inputs["W2"], f64)  # [S, 2, 3]
    b2 = np.asarray(inputs["b2"], f64)  # [S, 2]
    W3 = np.asarray(inputs["W3"], f64)  # [S, S, 2]
    b3 = np.asarray(inputs["b3"], f64)  # [S, S]
    idx = np.asarray(inputs["idx"], np.int64)  # [S, L]

    scale = gamma / np.sqrt(var + EPS)  # [S, L]
    shift = beta - mean * scale  # [S, L]

    # Weff[(s,o), f] = sum_l [idx[s,l]==f] W1[s,o,l]*scale[s,l]
    Wsc = W1 * scale[:, None, :]  # [S, 3, L]
    Weff = np.zeros((S, 3, FEAT), f64)
    s_ix = np.repeat(np.arange(S), 3 * L)
    o_ix = np.tile(np.repeat(np.arange(3), L), S)
    f_ix = np.repeat(idx[:, None, :], 3, axis=1).ravel()
    np.add.at(Weff, (s_ix, o_ix, f_ix), Wsc.ravel())
    Weff = Weff.reshape(3 * S, FEAT)
    beff = (b1 + np.einsum("sol,sl->so", W1, shift)).reshape(3 * S)

    # W2eff[(o2*S+s), (s*3+k)] = W2[s, o2, k]
    W2eff = np.zeros((2 * S, 3 * S), f64)
    for s in range(S):
        for o2 in range(2):
            W2eff[o2 * S + s, s * 3 : s * 3 + 3] = W2[s, o2, :]
    b2eff = b2.T.reshape(2 * S)
    W2effT = W2eff.T  # [300, 200]

    wt = np.zeros((128, WT_COLS), np.float16)
    WeffT = Weff.T  # [FEAT, 300]
    for k, c in enumerate(WT_WEFF):
        f0 = 128 * k
        fw = min(128, FEAT - f0)
        wt[0:fw, c : c + 300] = WeffT[f0 : f0 + fw, :]
    wt[407 - 384, WT_WEFF[3] : WT_WEFF[3] + 300] = beff  # ones col at x[:,407]
    for c_i, c in enumerate(WT_W2):
        wt[0:100, c : c + 200] = W2effT[c_i * 100 : (c_i + 1) * 100, :]
    wt[100, WT_W2[0] : WT_W2[0] + 200] = b2eff  # ones row of h1_0 carries b2
    # a-head variants: window w's 27 a-servers land at PSUM partitions
    # 101..127 (out col group 64, so slot r=37..63 maps to server 27w+r-37)
    for w in range(NWIN):
        for c_i in range(3):
            cv = WT_W2V + (w * 3 + c_i) * 64
            for r in range(37, 64):
                s = 27 * w + r - 37
                if s > 99:
                    break
                wt[0:100, cv + r] = W2effT[c_i * 100 : (c_i + 1) * 100, s]
                if c_i == 0:
                    wt[100, cv + r] = b2eff[s]  # o2=0 bias

    # mtc rows 0..99: [j, i*S+j] = W3[i,j,1] (one-hot in j)
    #     row 100:    b3 (vs the combined tile's ones row)
    #     row 101+r:  W3[i,j,0] where r = i - 27*window(col)
    mtc = np.zeros((128, SS), np.float16)
    g = np.arange(SS)
    i_g = g // S
    j_g = g % S
    w_g = np.minimum(g // WSPAN, NWIN - 1)
    mtc[j_g, g] = W3[i_g, j_g, 1].astype(np.float16)
    mtc[100, :] = b3.ravel()
    mtc[101 + (i_g - 27 * w_g), g] = W3[i_g, j_g, 0].astype(np.float16)

    return {"wt": wt, "mtc": mtc, "onesd": np.ones((1, BL), np.float16)}


def _build_module():
    global _module_cache
    if _module_cache is not None:
        return _module_cache

    nc = bacc.Bacc("TRN2", target_bir_lowering=False, debug=False, num_devices=N_CORES)
    xin = nc.dram_tensor("xin", [FP, BL], F16, kind="ExternalInput").ap()
    wt_d = nc.dram_tensor("wt", [128, WT_COLS], F16, kind="ExternalInput").ap()
    mtc_d = nc.dram_tensor("mtc", [128, SS], F16, kind="ExternalInput").ap()
    onesd = nc.dram_tensor("onesd", [1, BL], F16, kind="ExternalInput").ap()
    yout = nc.dram_tensor("yout", [BL, SS], F16, kind="ExternalOutput").ap()

    TANH = mybir.ActivationFunctionType.Tanh

    with TileContext(nc) as tc:
        with (
            tc.tile_pool(name="const", bufs=1) as const,
            tc.tile_pool(name="ot_pool", bufs=3) as ot_pool,
            tc.tile_pool(name="dve_pool", bufs=1) as dve_pool,
            tc.tile_pool(name="ps_pool", bufs=2, space="PSUM") as ps_pool,
        ):
            # ---- loads (gpsimd/SWDGE, full-128-partition: spreads engines) ----
            wt_t = const.tile([128, WT_COLS], F16)
            nc.gpsimd.dma_start(wt_t[:], wt_d[:, :])
            xT = []
            for k in range(4):
                xt = const.tile([128, BL], F16, name=f"xT_{k}", tag=f"xT{k}")
                nc.gpsimd.dma_start(xt[:], xin[128 * k : 128 * (k + 1), :])
                xT.append(xt)
            mtc_t = const.tile([128, SS], F16)
            for e0 in range(0, SS, SS // 2):
                nc.gpsimd.dma_start(
                    mtc_t[:, e0 : e0 + SS // 2], mtc_d[:, e0 : e0 + SS // 2]
                )

            h1 = []
            for m in range(3):
                rows = 101 if m == 0 else 100
                t = const.tile([rows, BL], F16, name=f"h1_{m}", tag=f"h1{m}")
                h1.append(t)
            comb = []
            for w in range(NWIN):
                t = const.tile([128, BL], F16, name=f"comb_{w}", tag=f"comb{w}")
                comb.append(t)
            nc.gpsimd.dma_start(h1[0][100:101, :], onesd[0:1, :])

            warm = const.tile([1, 8], F32)
            nc.scalar.activation(warm[:], wt_t[0:1, 0:8], TANH)  # tanh table preload

            # ---- front: h1 = tanh(x @ Weff.T + beff) ----
            for m in range(3):
                pm = ps_pool.tile([128, 2048], F32, name="pm", tag="ps")
                for h in range(2):
                    dst = pm[0:100, h * 512 : (h + 1) * 512]
                    for k in range(4):
                        nc.tensor.matmul(
                            dst,
                            wt_t[0:128, WT_WEFF[k] + 100 * m : WT_WEFF[k] + 100 * (m + 1)],
                            xT[k][0:128, h * 512 : (h + 1) * 512],
                            start=(k == 0),
                            stop=(k == 3),
                        )
                nc.scalar.activation(h1[m][0:100, :], pm[0:100, 0:BL], TANH)

            # ---- front: per-window combined tiles.  Window 0 completes
            # first so the final stage can start while windows 1..3 build.
            # Order per tile: a-head tanh into rows 96..127 (96..100 get
            # tanh(0)=0 from zero weight cols), THEN sb into 0..99 and ones
            # into row 100 — Tile subtile WAW deps enforce the overwrites.
            def emit_ta(w):
                ta = ps_pool.tile([128, 2048], F32, name="ta", tag="ps")
                for h in range(2):
                    dst = ta[64:128, h * 512 : (h + 1) * 512]
                    for c in range(3):
                        kr = 101 if c == 0 else 100
                        cv = WT_W2V + (w * 3 + c) * 64
                        nc.tensor.matmul(
                            dst,
                            wt_t[0:kr, cv : cv + 64],
                            h1[c][0:kr, h * 512 : (h + 1) * 512],
                            start=(c == 0),
                            stop=(c == 2),
                        )
                nc.scalar.activation(comb[w][96:128, :], ta[96:128, 0:BL], TANH)

            emit_ta(0)
            pm2 = ps_pool.tile([128, 2048], F32, name="pm2", tag="ps")
            for h in range(2):
                dst = pm2[0:100, h * 512 : (h + 1) * 512]
                for c in range(3):
                    kr = 101 if c == 0 else 100
                    nc.tensor.matmul(
                        dst,
                        wt_t[0:kr, WT_W2[c] + 100 : WT_W2[c] + 200],
                        h1[c][0:kr, h * 512 : (h + 1) * 512],
                        start=(c == 0),
                        stop=(c == 2),
                    )
            nc.scalar.activation(comb[0][0:100, :], pm2[0:100, 0:BL], TANH)
            nc.gpsimd.dma_start(comb[0][100:101, :], onesd[0:1, :])
            for w in range(1, NWIN):
                emit_ta(w)
                nc.vector.tensor_copy(comb[w][0:100, :], comb[0][0:100, :])
                nc.gpsimd.dma_start(comb[w][100:101, :], onesd[0:1, :])

            # ---- final: out = tanh(a_i w0 + sb_j w1 + b3), one matmul/segment ----
            for bs in range(BL // 128):
                ot = ot_pool.tile([128, SS], F16, name="ot", tag="ot")
                ca = bs * 128
                # q4 (the vector-engine cubic group) runs FIRST so its
                # 3-pass chain overlaps this block's scalar-engine groups —
                # except in block 0, which follows the front's window order
                # (q4 depends on window 3, the last combined tile built).
                for q in (0, 1, 2, 3, 4) if bs == 0 else (4, 0, 1, 2, 3):
                    qw = min(2048, SS - q * 2048)
                    pf = ps_pool.tile([128, 2048], F32, name="pf", tag="ps")
                    for c0, cw in SEGS:
                        if c0 // 2048 != q:
                            continue
                        w = min(c0 // WSPAN, NWIN - 1)
                        nc.tensor.matmul(
                            pf[0:128, c0 - q * 2048 : c0 - q * 2048 + cw],
                            comb[w][0:128, ca : ca + 128],
                            mtc_t[0:128, c0 : c0 + cw],
                            start=True,
                            stop=True,
                        )
                    if q < 4:
                        nc.scalar.activation(
                            ot[:, q * 2048 : q * 2048 + qw], pf[0:128, 0:qw], TANH
                        )
                    else:
                        # cubic tanh on the vector engine: z * (1 - z^2/3).
                        # One PSUM-reading copy so pf releases fast; the rest
                        # runs on SBUF.
                        z = pf[0:128, 0:qw]
                        zc = dve_pool.tile([128, 2048], F32, name="zc", tag="zc")
                        t2 = dve_pool.tile([128, 2048], F32, name="t2", tag="t2")
                        nc.vector.tensor_copy(zc[0:128, 0:qw], z)
                        nc.vector.scalar_tensor_tensor(
                            t2[0:128, 0:qw],
                            zc[0:128, 0:qw],
                            -1.0 / 3.0,
                            zc[0:128, 0:qw],
                            AluOpType.mult,
                            AluOpType.mult,
                        )
                        nc.vector.scalar_tensor_tensor(
                            ot[:, q * 2048 : q * 2048 + qw],
                            t2[0:128, 0:qw],
                            1.0,
                            zc[0:128, 0:qw],
                            AluOpType.add,
                            AluOpType.mult,
                        )
                    if q == 4:
                        nc.sync.dma_start(
                            yout[bs * 128 : (bs + 1) * 128, 8192:SS], ot[:, 8192:SS]
                        )
                    elif q == 1:
                        nc.sync.dma_start(
                            yout[bs * 128 : (bs + 1) * 128, 0:4096], ot[:, 0:4096]
                        )
                    elif q == 3:
                        nc.sync.dma_start(
                            yout[bs * 128 : (bs + 1) * 128, 4096:8192],
                            ot[:, 4096:8192],
                        )

    nc.compile()
    _module_cache = nc
    return nc


def _run(inputs, trace=False, trace_cores=None):
    nc = _build_module()
    hw = _host_weights(inputs)
    x = np.asarray(inputs["x"], np.float32)
    xpad = np.zeros((B, FP), np.float16)
    xpad[:, :FEAT] = x.astype(np.float16)
    xpad[:, FEAT] = 1.0
    in_maps = []
    for c in range(N_CORES):
        m = dict(hw)
        m["xin"] = np.ascontiguousarray(xpad[c * BL : (c + 1) * BL].T)
        in_maps.append(m)
    kwargs = {}
    if trace:
        bass_utils.upload_artifacts = lambda tmpdir: tmpdir  # no cloud store here
        kwargs = dict(trace=True, trace_cores=trace_cores or [0])
    res = bass_utils.run_bass_kernel_spmd(
        nc, in_maps, core_ids=list(range(N_CORES)), **kwargs
    )
    out = np.concatenate(
        [res.results[c]["yout"] for c in range(N_CORES)], axis=0
    ).astype(np.float32)
    return out, res


def kernel(**inputs) -> np.ndarray:
    out, _ = _run(inputs)
    return out


# revision 50
# speedup vs baseline: 1.0445x; 1.0445x over previous
"""Trainium2 Bass kernel for nn_DQN_57904749085018 (gnn_message_passing).

Computation (reference semantics):
    g   = x[:, idx]                                  [B, S, L] gather
    h   = (g - mean) * rsqrt(var+eps) * gamma + beta [B, S, L] batchnorm (eval)
    h1  = tanh(einsum('bsl,sol->bso', h, W1) + b1)   [B, S, 3]
    h2  = tanh(einsum('bsk,sok->bso', h1, W2) + b2)  [B, S, 2]
    a, sb = h2[..., 0], h2[..., 1]
    out[b,i,j] = tanh(a[b,i]*W3[i,j,0] + sb[b,j]*W3[i,j,1] + b3[i,j])
    -> reshape [B, S*S]

Kernel strategy (pure data parallel over 8 cores, batch-sharded), fp16
end-to-end (measured rel err ~5e-4 against the fp64 oracle, tolerance 2e-2):
  * gather + batchnorm + Linear1 fold into one dense matmul x @ Weff.T;
    x is padded to 512 features with a ones column at 407 so the biases
    ride along as ordinary weight rows.  x is transposed on the host.
  * the pairwise head out[b,(i,j)] = tanh(a_i w0_ij + sb_j w1_ij + b_ij)
    runs as a SINGLE K=128 matmul per output span: the output columns are
    split into four 2700-col windows (27 consecutive i-servers each), and
    per window a combined stationary tile holds sb (rows 0..99), a ones
    row carrying b3 (row 100), and the window's 27 a-rows (rows 101..127).
    The a-rows land at partitions 101..127 directly out of the W2 head by
    computing that head with a column-rearranged stationary into PSUM
    partitions 96..127 (no cross-partition copies exist on any engine).
    The matching table packs W3[:,:,1] (one-hot in j), b3, and the
    windowed W3[:,:,0] rows into one [128, S*S] fp16 operand.
  * the final tanh runs on the scalar engine for output cols 0..8191 and
    as a cubic `z - z^3/3` on the (otherwise idle) vector engine for cols
    8192..9999 (|z| <= 3/sqrt(300) = 0.174 so the cubic errs < 2.2e-5),
    removing the scalar engine as the sole steady-state bottleneck.
  * output is written as fp16 (half the HBM write traffic) in three
    contiguous chunks per 128-row block on the sync ring; the host
    upcasts to fp32.
  * all bulk loads go through gpsimd (SWDGE) as full-128-partition
    transfers: HWDGE-dynamic and partial-partition loads clump onto one
    or two SDMA engines (~27 GiB/s) instead of spreading across all 16.
"""

import sys

import numpy as np

if "/opt/trn_rl_repo" not in sys.path:
    sys.path.insert(0, "/opt/trn_rl_repo")

import concourse.bacc as bacc
import concourse.mybir as mybir
from concourse import bass_utils
from concourse.alu_op_type import AluOpType
from concourse.tile import TileContext

S = 100
L = 13
FEAT = 4 * S + 7  # 407
FP = 512  # padded feature width (col 407 = ones, 408.. = zero)
B = 8192
EPS = 1e-5
N_CORES = 8
BL = B // N_CORES  # 1024 batch rows per core
SS = S * S  # 10000
WSPAN = 2700  # output window: 27 consecutive i-servers
NWIN = 4  # windows 0..2 cover 2700 cols, window 3 covers 1900
F16 = mybir.dt.float16
F32 = mybir.dt.float32

# wt pack layout (columns of the [128, WT_COLS] fp16 tile)
WT_WEFF = [0, 300, 600, 900]  # WeffT feature-chunk k at col 300k, width 300
WT_W2 = [1200, 1400, 1600]  # W2effT k-chunk c, width 200 (cols 100..199 = sb)
WT_W2V = 1800  # 12 blocks of 64: a-head variant for window w, chunk c
WT_COLS = 1800 + NWIN * 3 * 64  # 2568

# final-stage matmul segments: 512-col PSUM-bank grid split at window edges
_EDGES = sorted(set(list(range(0, SS, 512)) + [WSPAN, 2 * WSPAN, 3 * WSPAN, SS]))
SEGS = [(a, b - a) for a, b in zip(_EDGES, _EDGES[1:])]

_module_cache = None


def _build_indices():
    idx = [[2 * i, 2 * i + 1] for i in range(S)]
    start = 2 * S
    for k in range(S):
        u, v = k, (k + 1) % S
        idx[u].extend([start, start + 1])
        idx[v].extend([start, start + 1])
        start += 2
    g0 = 4 * S
    for i in range(S):
        idx[i].extend(range(g0, g0 + 7))
    return np.asarray(idx, dtype=np.int64)


def _host_weights(inputs):
    f64 = np.float64
    gamma = np.asarray(inputs["gamma"], f64)
    beta = np.asarray(inputs["beta"], f64)
    mean = np.asarray(inputs["mean"], f64)
    var = np.asarray(inputs["var"], f64)
    W1 = np.asarray(inputs["W1"], f64)  # [S, 3, L]
    b1 = np.asarray(inputs["b1"], f64)  # [S, 3]
    W2 = np.asarray(inputs["W2"], f64)  # [S, 2, 3]
    b2 = np.asarray(inputs["b2"], f64)  # [S, 2]
    W3 = np.asarray(inputs["W3"], f64)  # [S, S, 2]
    b3 = np.asarray(inputs["b3"], f64)  # [S, S]
    idx = np.asarray(inputs["idx"], np.int64)  # [S, L]

    scale = gamma / np.sqrt(var + EPS)  # [S, L]
    shift = beta - mean * scale  # [S, L]

    # Weff[(s,o), f] = sum_l [idx[s,l]==f] W1[s,o,l]*scale[s,l]
    Wsc = W1 * scale[:, None, :]  # [S, 3, L]
    Weff = np.zeros((S, 3, FEAT), f64)
    s_ix = np.repeat(np.arange(S), 3 * L)
    o_ix = np.tile(np.repeat(np.arange(3), L), S)
    f_ix = np.repeat(idx[:, None, :], 3, axis=1).ravel()
    np.add.at(Weff, (s_ix, o_ix, f_ix), Wsc.ravel())
    Weff = Weff.reshape(3 * S, FEAT)
    beff = (b1 + np.einsum("sol,sl->so", W1, shift)).reshape(3 * S)

    # W2eff[(o2*S+s), (s*3+k)] = W2[s, o2, k]
    W2eff = np.zeros((2 * S, 3 * S), f64)
    for s in range(S):
        for o2 in range(2):
            W2eff[o2 * S + s, s * 3 : s * 3 + 3] = W2[s, o2, :]
    b2eff = b2.T.reshape(2 * S)
    W2effT = W2eff.T  # [300, 200]

    wt = np.zeros((128, WT_COLS), np.float16)
    WeffT = Weff.T  # [FEAT, 300]
    for k, c in enumerate(WT_WEFF):
        f0 = 128 * k
        fw = min(128, FEAT - f0)
        wt[0:fw, c : c + 300] = WeffT[f0 : f0 + fw, :]
    wt[407 - 384, WT_WEFF[3] : WT_WEFF[3] + 300] = beff  # ones col at x[:,407]
    for c_i, c in enumerate(WT_W2):
        wt[0:100, c : c + 200] = W2effT[c_i * 100 : (c_i + 1) * 100, :]
    wt[100, WT_W2[0] : WT_W2[0] + 200] = b2eff  # ones row of h1_0 carries b2
    # a-head variants: window w's 27 a-servers land at PSUM partitions
    # 101..127 (out col group 64, so slot r=37..63 maps to server 27w+r-37)
    for w in range(NWIN):
        for c_i in range(3):
            cv = WT_W2V + (w * 3 + c_i) * 64
            for r in range(37, 64):
                s = 27 * w + r - 37
                if s > 99:
                    break
                wt[0:100, cv + r] = W2effT[c_i * 100 : (c_i + 1) * 100, s]
                if c_i == 0:
                    wt[100, cv + r] = b2eff[s]  # o2=0 bias

    # mtc rows 0..99: [j, i*S+j] = W3[i,j,1] (one-hot in j)
    #     row 100:    b3 (vs the combined tile's ones row)
    #     row 101+r:  W3[i,j,0] where r = i - 27*window(col)
    mtc = np.zeros((128, SS), np.float16)
    g = np.arange(SS)
    i_g = g // S
    j_g = g % S
    w_g = np.minimum(g // WSPAN, NWIN - 1)
    mtc[j_g, g] = W3[i_g, j_g, 1].astype(np.float16)
    mtc[100, :] = b3.ravel()
    mtc[101 + (i_g - 27 * w_g), g] = W3[i_g, j_g, 0].astype(np.float16)

    return {"wt": wt, "mtc": mtc, "onesd": np.ones((1, BL), np.float16)}


def _build_module():
    global _module_cache
    if _module_cache is not None:
        return _module_cache

    nc = bacc.Bacc("TRN2", target_bir_lowering=False, debug=False, num_devices=N_CORES)
    xin = nc.dram_tensor("xin", [FP, BL], F16, kind="ExternalInput").ap()
    wt_d = nc.dram_tensor("wt", [128, WT_COLS], F16, kind="ExternalInput").ap()
    mtc_d = nc.dram_tensor("mtc", [128, SS], F16, kind="ExternalInput").ap()
    onesd = nc.dram_tensor("onesd", [1, BL], F16, kind="ExternalInput").ap()
    yout = nc.dram_tensor("yout", [BL, SS], F16, kind="ExternalOutput").ap()

    TANH = mybir.ActivationFunctionType.Tanh

    with TileContext(nc) as tc:
        with (
            tc.tile_pool(name="const", bufs=1) as const,
            tc.tile_pool(name="ot_pool", bufs=3) as ot_pool,
            tc.tile_pool(name="dve_pool", bufs=1) as dve_pool,
            tc.tile_pool(name="ps_pool", bufs=2, space="PSUM") as ps_pool,
        ):
            # ---- loads (gpsimd/SWDGE, full-128-partition: spreads engines) ----
            wt_t = const.tile([128, WT_COLS], F16)
            nc.gpsimd.dma_start(wt_t[:], wt_d[:, :])
            xT = []
            for k in range(4):
                xt = const.tile([128, BL], F16, name=f"xT_{k}", tag=f"xT{k}")
                nc.gpsimd.dma_start(xt[:], xin[128 * k : 128 * (k + 1), :])
                xT.append(xt)
            mtc_t = const.tile([128, SS], F16)
            for e0 in range(0, SS, SS // 2):
                nc.gpsimd.dma_start(
                    mtc_t[:, e0 : e0 + SS // 2], mtc_d[:, e0 : e0 + SS // 2]
                )

            h1 = []
            for m in range(3):
                rows = 101 if m == 0 else 100
                t = const.tile([rows, BL], F16, name=f"h1_{m}", tag=f"h1{m}")
                h1.append(t)
            comb = []
            for w in range(NWIN):
                t = const.tile([128, BL], F16, name=f"comb_{w}", tag=f"comb{w}")
                comb.append(t)
            nc.gpsimd.dma_start(h1[0][100:101, :], onesd[0:1, :])

            warm = const.tile([1, 8], F32)
            nc.scalar.activation(warm[:], wt_t[0:1, 0:8], TANH)  # tanh table preload

            # ---- front: h1 = tanh(x @ Weff.T + beff) ----
            for m in range(3):
                pm = ps_pool.tile([128, 2048], F32, name="pm", tag="ps")
                for h in range(2):
                    dst = pm[0:100, h * 512 : (h + 1) * 512]
                    for k in range(4):
                        nc.tensor.matmul(
                            dst,
                            wt_t[0:128, WT_WEFF[k] + 100 * m : WT_WEFF[k] + 100 * (m + 1)],
                            xT[k][0:128, h * 512 : (h + 1) * 512],
                            start=(k == 0),
                            stop=(k == 3),
                        )
                nc.scalar.activation(h1[m][0:100, :], pm[0:100, 0:BL], TANH)

            # ---- front: per-window combined tiles.  Window 0 completes
            # first so the final stage can start while windows 1..3 build.
            # Order per tile: a-head tanh into rows 96..127 (96..100 get
            # tanh(0)=0 from zero weight cols), THEN sb into 0..99 and ones
            # into row 100 — Tile subtile WAW deps enforce the overwrites.
            def emit_ta(w):
                ta = ps_pool.tile([128, 2048], F32, name="ta", tag="ps")
                for h in range(2):
                    dst = ta[64:128, h * 512 : (h + 1) * 512]
                    for c in range(3):
                        kr = 101 if c == 0 else 100
                        cv = WT_W2V + (w * 3 + c) * 64
                        nc.tensor.matmul(
                            dst,
                            wt_t[0:kr, cv : cv + 64],
                            h1[c][0:kr, h * 512 : (h + 1) * 512],
                            start=(c == 0),
                            stop=(c == 2),
                        )
                nc.scalar.activation(comb[w][96:128, :], ta[96:128, 0:BL], TANH)

            emit_ta(0)
            pm2 = ps_pool.tile([128, 2048], F32, name="pm2", tag="ps")
            for h in range(2):
                dst = pm2[0:100, h * 512 : (h + 1) * 512]
                for c in range(3):
                    kr = 101 if c == 0 else 100
                    nc.tensor.matmul(
                        dst,
                        wt_t[0:kr, WT_W2[c] + 100 : WT_W2[c] + 200],
                        h1[c][0:kr, h * 512 : (h + 1) * 512],
                        start=(c == 0),
                        stop=(c == 2),
                    )
            nc.scalar.activation(comb[0][0:100, :], pm2[0:100, 0:BL], TANH)
            nc.gpsimd.dma_start(comb[0][100:101, :], onesd[0:1, :])
            for w in range(1, NWIN):
                emit_ta(w)
                nc.vector.tensor_copy(comb[w][0:100, :], comb[0][0:100, :])
                nc.gpsimd.dma_start(comb[w][100:101, :], onesd[0:1, :])

            # ---- final: out = tanh(a_i w0 + sb_j w1 + b3), one matmul/segment ----
            for bs in range(BL // 128):
                ot = ot_pool.tile([128, SS], F16, name="ot", tag="ot")
                ca = bs * 128
                # q4 (the vector-engine cubic group) runs FIRST so its
                # 3-pass chain overlaps this block's scalar-engine groups —
                # except in block 0, which follows the front's window order
                # (q4 depends on window 3, the last combined tile built).
                for q in (0, 1, 2, 3, 4) if bs == 0 else (4, 0, 1, 2, 3):
                    qw = min(2048, SS - q * 2048)
                    pf = ps_pool.tile([128, 2048], F32, name="pf", tag="ps")
                    for c0, cw in SEGS:
                        if c0 // 2048 != q:
                            continue
                        w = min(c0 // WSPAN, NWIN - 1)
                        nc.tensor.matmul(
                            pf[0:128, c0 - q * 2048 : c0 - q * 2048 + cw],
                            comb[w][0:128, ca : ca + 128],
                            mtc_t[0:128, c0 : c0 + cw],
                            start=True,
                            stop=True,
                        )
                    if q < 4:
                        # q3's last 256 cols go to the vector-engine cubic to
                        # balance scalar vs vector engine load per block.
                        aw = 2048 if q < 3 else 1792
                        nc.scalar.activation(
                            ot[:, q * 2048 : q * 2048 + aw], pf[0:128, 0:aw], TANH
                        )
                        if q == 3:
                            zc3 = dve_pool.tile(
                                [128, 256], F32, name="zc3", tag="zc3"
                            )
                            t23 = dve_pool.tile(
                                [128, 256], F32, name="t23", tag="t23"
                            )
                            nc.vector.tensor_copy(zc3[:, :], pf[0:128, 1792:2048])
                            nc.vector.scalar_tensor_tensor(
                                t23[:, :], zc3[:, :], -1.0 / 3.0, zc3[:, :],
                                AluOpType.mult, AluOpType.mult,
                            )
                            nc.vector.scalar_tensor_tensor(
                                ot[:, 7936:8192], t23[:, :], 1.0, zc3[:, :],
                                AluOpType.add, AluOpType.mult,
                            )
                    else:
                        # cubic tanh on the vector engine: z * (1 - z^2/3).
                        # One PSUM-reading copy so pf releases fast; the rest
                        # runs on SBUF.
                        z = pf[0:128, 0:qw]
                        zc = dve_pool.tile([128, 2048], F32, name="zc", tag="zc")
                        t2 = dve_pool.tile([128, 2048], F32, name="t2", tag="t2")
                        nc.vector.tensor_copy(zc[0:128, 0:qw], z)
                        nc.vector.scalar_tensor_tensor(
                            t2[0:128, 0:qw],
                            zc[0:128, 0:qw],
                            -1.0 / 3.0,
                            zc[0:128, 0:qw],
                            AluOpType.mult,
                            AluOpType.mult,
                        )
                        nc.vector.scalar_tensor_tensor(
                            ot[:, q * 2048 : q * 2048 + qw],
                            t2[0:128, 0:qw],
                            1.0,
                            zc[0:128, 0:qw],
                            AluOpType.add,
                            AluOpType.mult,
                        )
                    if q == 4:
                        nc.sync.dma_start(
                            yout[bs * 128 : (bs + 1) * 128, 8192:SS], ot[:, 8192:SS]
                        )
                    elif q == 1:
                        nc.sync.dma_start(
                            yout[bs * 128 : (bs + 1) * 128, 0:4096], ot[:, 0:4096]
                        )
                    elif q == 3:
                        nc.sync.dma_start(
                            yout[bs * 128 : (bs + 1) * 128, 4096:8192],
                            ot[:, 4096:8192],
                        )

    nc.compile()
    _module_cache = nc
    return nc


def _run(inputs, trace=False, trace_cores=None):
    nc = _build_module()
    hw = _host_weights(inputs)
    x = np.asarray(inputs["x"], np.float32)
    xpad = np.zeros((B, FP), np.float16)
    xpad[:, :FEAT] = x.astype(np.float16)
    xpad[:, FEAT] = 1.0
    in_maps = []
    for c in range(N_CORES):
        m = dict(hw)
        m["xin"] = np.ascontiguousarray(xpad[c * BL : (c + 1) * BL].T)
        in_maps.append(m)
    kwargs = {}
    if trace:
        bass_utils.upload_artifacts = lambda tmpdir: tmpdir  # no cloud store here
        kwargs = dict(trace=True, trace_cores=trace_cores or [0])
    res = bass_utils.run_bass_kernel_spmd(
        nc, in_maps, core_ids=list(range(N_CORES)), **kwargs
    )
    out = np.concatenate(
        [res.results[c]["yout"] for c in range(N_CORES)], axis=0
    ).astype(np.float32)
    return out, res


def kernel(**inputs) -> np.ndarray:
    out, _ = _run(inputs)
    return out


# revision 51
# speedup vs baseline: 1.1379x; 1.0895x over previous
"""Trainium2 Bass kernel for nn_DQN_57904749085018 (gnn_message_passing).

Computation (reference semantics):
    g   = x[:, idx]                                  [B, S, L] gather
    h   = (g - mean) * rsqrt(var+eps) * gamma + beta [B, S, L] batchnorm (eval)
    h1  = tanh(einsum('bsl,sol->bso', h, W1) + b1)   [B, S, 3]
    h2  = tanh(einsum('bsk,sok->bso', h1, W2) + b2)  [B, S, 2]
    a, sb = h2[..., 0], h2[..., 1]
    out[b,i,j] = tanh(a[b,i]*W3[i,j,0] + sb[b,j]*W3[i,j,1] + b3[i,j])
    -> reshape [B, S*S]

Kernel strategy (pure data parallel over 8 cores, batch-sharded), fp16
end-to-end (measured rel err ~5e-4 against the fp64 oracle, tolerance 2e-2):
  * gather + batchnorm + Linear1 fold into one dense matmul x @ Weff.T;
    x is padded to 512 features with a ones column at 407 so the biases
    ride along as ordinary weight rows.  x is transposed on the host.
  * the pairwise head out[b,(i,j)] = tanh(a_i w0_ij + sb_j w1_ij + b_ij)
    runs as a SINGLE K=128 matmul per output span: the output columns are
    split into four 2700-col windows (27 consecutive i-servers each), and
    per window a combined stationary tile holds sb (rows 0..99), a ones
    row carrying b3 (row 100), and the window's 27 a-rows (rows 101..127).
    The a-rows land at partitions 101..127 directly out of the W2 head by
    computing that head with a column-rearranged stationary into PSUM
    partitions 96..127 (no cross-partition copies exist on any engine).
    The matching table packs W3[:,:,1] (one-hot in j), b3, and the
    windowed W3[:,:,0] rows into one [128, S*S] fp16 operand.
  * the final tanh runs on the scalar engine for output cols 0..8191 and
    as a cubic `z - z^3/3` on the (otherwise idle) vector engine for cols
    8192..9999 (|z| <= 3/sqrt(300) = 0.174 so the cubic errs < 2.2e-5),
    removing the scalar engine as the sole steady-state bottleneck.
  * output is written as fp16 (half the HBM write traffic) in three
    contiguous chunks per 128-row block on the sync ring; the host
    upcasts to fp32.
  * all bulk loads go through gpsimd (SWDGE) as full-128-partition
    transfers: HWDGE-dynamic and partial-partition loads clump onto one
    or two SDMA engines (~27 GiB/s) instead of spreading across all 16.
"""

import sys

import numpy as np

if "/opt/trn_rl_repo" not in sys.path:
    sys.path.insert(0, "/opt/trn_rl_repo")

import concourse.bacc as bacc
import concourse.mybir as mybir
from concourse import bass_utils
from concourse.alu_op_type import AluOpType
from concourse.tile import TileContext

S = 100
L = 13
FEAT = 4 * S + 7  # 407
FP = 512  # padded feature width (col 407 = ones, 408.. = zero)
B = 8192
EPS = 1e-5
N_CORES = 8
BL = B // N_CORES  # 1024 batch rows per core
SS = S * S  # 10000
WSPAN = 2700  # output window: 27 consecutive i-servers
NWIN = 4  # windows 0..2 cover 2700 cols, window 3 covers 1900
F16 = mybir.dt.float16
F32 = mybir.dt.float32

# wt pack layout (columns of the [128, WT_COLS] fp16 tile)
WT_WEFF = [0, 300, 600, 900]  # WeffT feature-chunk k at col 300k, width 300
WT_W2 = [1200, 1400, 1600]  # W2effT k-chunk c, width 200 (cols 100..199 = sb)
WT_W2V = 1800  # 12 blocks of 64: a-head variant for window w, chunk c
WT_COLS = 1800 + NWIN * 3 * 64  # 2568

# final-stage matmul segments: 512-col PSUM-bank grid split at window edges
_EDGES = sorted(set(list(range(0, SS, 512)) + [WSPAN, 2 * WSPAN, 3 * WSPAN, SS]))
SEGS = [(a, b - a) for a, b in zip(_EDGES, _EDGES[1:])]

_module_cache = None


def _build_indices():
    idx = [[2 * i, 2 * i + 1] for i in range(S)]
    start = 2 * S
    for k in range(S):
        u, v = k, (k + 1) % S
        idx[u].extend([start, start + 1])
        idx[v].extend([start, start + 1])
        start += 2
    g0 = 4 * S
    for i in range(S):
        idx[i].extend(range(g0, g0 + 7))
    return np.asarray(idx, dtype=np.int64)


def _host_weights(inputs):
    f64 = np.float64
    gamma = np.asarray(inputs["gamma"], f64)
    beta = np.asarray(inputs["beta"], f64)
    mean = np.asarray(inputs["mean"], f64)
    var = np.asarray(inputs["var"], f64)
    W1 = np.asarray(inputs["W1"], f64)  # [S, 3, L]
    b1 = np.asarray(inputs["b1"], f64)  # [S, 3]
    W2 = np.asarray(inputs["W2"], f64)  # [S, 2, 3]
    b2 = np.asarray(inputs["b2"], f64)  # [S, 2]
    W3 = np.asarray(inputs["W3"], f64)  # [S, S, 2]
    b3 = np.asarray(inputs["b3"], f64)  # [S, S]
    idx = np.asarray(inputs["idx"], np.int64)  # [S, L]

    scale = gamma / np.sqrt(var + EPS)  # [S, L]
    shift = beta - mean * scale  # [S, L]

    # Weff[(s,o), f] = sum_l [idx[s,l]==f] W1[s,o,l]*scale[s,l]
    Wsc = W1 * scale[:, None, :]  # [S, 3, L]
    Weff = np.zeros((S, 3, FEAT), f64)
    s_ix = np.repeat(np.arange(S), 3 * L)
    o_ix = np.tile(np.repeat(np.arange(3), L), S)
    f_ix = np.repeat(idx[:, None, :], 3, axis=1).ravel()
    np.add.at(Weff, (s_ix, o_ix, f_ix), Wsc.ravel())
    Weff = Weff.reshape(3 * S, FEAT)
    beff = (b1 + np.einsum("sol,sl->so", W1, shift)).reshape(3 * S)

    # W2eff[(o2*S+s), (s*3+k)] = W2[s, o2, k]
    W2eff = np.zeros((2 * S, 3 * S), f64)
    for s in range(S):
        for o2 in range(2):
            W2eff[o2 * S + s, s * 3 : s * 3 + 3] = W2[s, o2, :]
    b2eff = b2.T.reshape(2 * S)
    W2effT = W2eff.T  # [300, 200]

    wt = np.zeros((128, WT_COLS), np.float16)
    WeffT = Weff.T  # [FEAT, 300]
    for k, c in enumerate(WT_WEFF):
        f0 = 128 * k
        fw = min(128, FEAT - f0)
        wt[0:fw, c : c + 300] = WeffT[f0 : f0 + fw, :]
    wt[407 - 384, WT_WEFF[3] : WT_WEFF[3] + 300] = beff  # ones col at x[:,407]
    for c_i, c in enumerate(WT_W2):
        wt[0:100, c : c + 200] = W2effT[c_i * 100 : (c_i + 1) * 100, :]
    wt[100, WT_W2[0] : WT_W2[0] + 200] = b2eff  # ones row of h1_0 carries b2
    # a-head variants: window w's 27 a-servers land at PSUM partitions
    # 101..127 (out col group 64, so slot r=37..63 maps to server 27w+r-37)
    for w in range(NWIN):
        for c_i in range(3):
            cv = WT_W2V + (w * 3 + c_i) * 64
            for r in range(37, 64):
                s = 27 * w + r - 37
                if s > 99:
                    break
                wt[0:100, cv + r] = W2effT[c_i * 100 : (c_i + 1) * 100, s]
                if c_i == 0:
                    wt[100, cv + r] = b2eff[s]  # o2=0 bias

    # mtc rows 0..99: [j, i*S+j] = W3[i,j,1] (one-hot in j)
    #     row 100:    b3 (vs the combined tile's ones row)
    #     row 101+r:  W3[i,j,0] where r = i - 27*window(col)
    mtc = np.zeros((128, SS), np.float16)
    g = np.arange(SS)
    i_g = g // S
    j_g = g % S
    w_g = np.minimum(g // WSPAN, NWIN - 1)
    mtc[j_g, g] = W3[i_g, j_g, 1].astype(np.float16)
    mtc[100, :] = b3.ravel()
    mtc[101 + (i_g - 27 * w_g), g] = W3[i_g, j_g, 0].astype(np.float16)

    return {"wt": wt, "mtc": mtc, "onesd": np.ones((1, BL), np.float16)}


def _build_module():
    global _module_cache
    if _module_cache is not None:
        return _module_cache

    nc = bacc.Bacc("TRN2", target_bir_lowering=False, debug=False, num_devices=N_CORES)
    xin = nc.dram_tensor("xin", [FP, BL], F16, kind="ExternalInput").ap()
    wt_d = nc.dram_tensor("wt", [128, WT_COLS], F16, kind="ExternalInput").ap()
    mtc_d = nc.dram_tensor("mtc", [128, SS], F16, kind="ExternalInput").ap()
    onesd = nc.dram_tensor("onesd", [1, BL], F16, kind="ExternalInput").ap()
    yout = nc.dram_tensor("yout", [BL, SS], F16, kind="ExternalOutput").ap()

    TANH = mybir.ActivationFunctionType.Tanh

    with TileContext(nc) as tc:
        with (
            tc.tile_pool(name="const", bufs=1) as const,
            tc.tile_pool(name="ot_pool", bufs=3) as ot_pool,
            tc.tile_pool(name="dve_pool", bufs=1) as dve_pool,
            tc.tile_pool(name="ps_pool", bufs=2, space="PSUM") as ps_pool,
        ):
            # ---- loads (gpsimd/SWDGE, full-128-partition: spreads engines) ----
            wt_t = const.tile([128, WT_COLS], F16)
            nc.gpsimd.dma_start(wt_t[:], wt_d[:, :])
            xT = []
            for k in range(4):
                xt = const.tile([128, BL], F16, name=f"xT_{k}", tag=f"xT{k}")
                nc.gpsimd.dma_start(xt[:], xin[128 * k : 128 * (k + 1), :])
                xT.append(xt)
            mtc_t = const.tile([128, SS], F16)
            for e0 in range(0, SS, SS // 2):
                nc.gpsimd.dma_start(
                    mtc_t[:, e0 : e0 + SS // 2], mtc_d[:, e0 : e0 + SS // 2]
                )

            h1 = []
            for m in range(3):
                rows = 101 if m == 0 else 100
                t = const.tile([rows, BL], F16, name=f"h1_{m}", tag=f"h1{m}")
                h1.append(t)
            comb = []
            for w in range(NWIN):
                t = const.tile([128, BL], F16, name=f"comb_{w}", tag=f"comb{w}")
                comb.append(t)
            nc.gpsimd.dma_start(h1[0][100:101, :], onesd[0:1, :])

            warm = const.tile([1, 8], F32)
            nc.scalar.activation(warm[:], wt_t[0:1, 0:8], TANH)  # tanh table preload

            # ---- front: h1 = tanh(x @ Weff.T + beff) ----
            for m in range(3):
                pm = ps_pool.tile([128, 2048], F32, name="pm", tag="ps")
                for h in range(2):
                    dst = pm[0:100, h * 512 : (h + 1) * 512]
                    for k in range(4):
                        nc.tensor.matmul(
                            dst,
                            wt_t[0:128, WT_WEFF[k] + 100 * m : WT_WEFF[k] + 100 * (m + 1)],
                            xT[k][0:128, h * 512 : (h + 1) * 512],
                            start=(k == 0),
                            stop=(k == 3),
                        )
                nc.scalar.activation(h1[m][0:100, :], pm[0:100, 0:BL], TANH)

            # ---- front: per-window combined tiles.  Window 0 completes
            # first so the final stage can start while windows 1..3 build.
            # Order per tile: a-head tanh into rows 96..127 (96..100 get
            # tanh(0)=0 from zero weight cols), THEN sb into 0..99 and ones
            # into row 100 — Tile subtile WAW deps enforce the overwrites.
            def emit_ta(w):
                ta = ps_pool.tile([128, 2048], F32, name="ta", tag="ps")
                for h in range(2):
                    dst = ta[64:128, h * 512 : (h + 1) * 512]
                    for c in range(3):
                        kr = 101 if c == 0 else 100
                        cv = WT_W2V + (w * 3 + c) * 64
                        nc.tensor.matmul(
                            dst,
                            wt_t[0:kr, cv : cv + 64],
                            h1[c][0:kr, h * 512 : (h + 1) * 512],
                            start=(c == 0),
                            stop=(c == 2),
                        )
                nc.scalar.activation(comb[w][96:128, :], ta[96:128, 0:BL], TANH)

            emit_ta(0)
            pm2 = ps_pool.tile([128, 2048], F32, name="pm2", tag="ps")
            for h in range(2):
                dst = pm2[0:100, h * 512 : (h + 1) * 512]
                for c in range(3):
                    kr = 101 if c == 0 else 100
                    nc.tensor.matmul(
                        dst,
                        wt_t[0:kr, WT_W2[c] + 100 : WT_W2[c] + 200],
                        h1[c][0:kr, h * 512 : (h + 1) * 512],
                        start=(c == 0),
                        stop=(c == 2),
                    )
            nc.scalar.activation(comb[0][0:100, :], pm2[0:100, 0:BL], TANH)
            nc.gpsimd.dma_start(comb[0][100:101, :], onesd[0:1, :])
            for w in range(1, NWIN):
                emit_ta(w)
                nc.vector.tensor_copy(comb[w][0:100, :], comb[0][0:100, :])
                nc.gpsimd.dma_start(comb[w][100:101, :], onesd[0:1, :])

            # ---- final: out = tanh(a_i w0 + sb_j w1 + b3), one matmul/segment ----
            for bs in range(BL // 128):
                ot = ot_pool.tile([128, SS], F16, name="ot", tag="ot")
                ca = bs * 128
                # q4 (the vector-engine cubic group) runs FIRST so its
                # 3-pass chain overlaps this block's scalar-engine groups —
                # except in block 0, which follows the front's window order
                # (q4 depends on window 3, the last combined tile built).
                for q in (0, 1, 2, 3, 4) if bs == 0 else (4, 0, 1, 2, 3):
                    qw = min(2048, SS - q * 2048)
                    pf = ps_pool.tile([128, 2048], F32, name="pf", tag="ps")
                    for c0, cw in SEGS:
                        if c0 // 2048 != q:
                            continue
                        w = min(c0 // WSPAN, NWIN - 1)
                        nc.tensor.matmul(
                            pf[0:128, c0 - q * 2048 : c0 - q * 2048 + cw],
                            comb[w][0:128, ca : ca + 128],
                            mtc_t[0:128, c0 : c0 + cw],
                            start=True,
                            stop=True,
                        )
                    if q < 4:
                        nc.scalar.activation(
                            ot[:, q * 2048 : q * 2048 + qw], pf[0:128, 0:qw], TANH
                        )
                    else:
                        # cubic tanh on the vector engine: z * (1 - z^2/3).
                        # One PSUM-reading copy so pf releases fast; the rest
                        # runs on SBUF.
                        z = pf[0:128, 0:qw]
                        zc = dve_pool.tile([128, 2048], F32, name="zc", tag="zc")
                        t2 = dve_pool.tile([128, 2048], F32, name="t2", tag="t2")
                        nc.vector.tensor_copy(zc[0:128, 0:qw], z)
                        nc.vector.scalar_tensor_tensor(
                            t2[0:128, 0:qw],
                            zc[0:128, 0:qw],
                            -1.0 / 3.0,
                            zc[0:128, 0:qw],
                            AluOpType.mult,
                            AluOpType.mult,
                        )
                        nc.vector.scalar_tensor_tensor(
                            ot[:, q * 2048 : q * 2048 + qw],
                            t2[0:128, 0:qw],
                            1.0,
                            zc[0:128, 0:qw],
                            AluOpType.add,
                            AluOpType.mult,
                        )
                    if q == 4:
                        nc.sync.dma_start(
                            yout[bs * 128 : (bs + 1) * 128, 8192:SS], ot[:, 8192:SS]
                        )
                    elif q == 1:
                        nc.sync.dma_start(
                            yout[bs * 128 : (bs + 1) * 128, 0:4096], ot[:, 0:4096]
                        )
                    elif q == 3:
                        nc.sync.dma_start(
                            yout[bs * 128 : (bs + 1) * 128, 4096:8192],
                            ot[:, 4096:8192],
                        )

    nc.compile()
    _module_cache = nc
    return nc


def _run(inputs, trace=False, trace_cores=None):
    nc = _build_module()
    hw = _host_weights(inputs)
    x = np.asarray(inputs["x"], np.float32)
    xpad = np.zeros((B, FP), np.float16)
    xpad[:, :FEAT] = x.astype(np.float16)
    xpad[:, FEAT] = 1.0
    in_maps = []
    for c in range(N_CORES):
        m = dict(hw)
        m["xin"] = np.ascontiguousarray(xpad[c * BL : (c + 1) * BL].T)
        in_maps.append(m)
    kwargs = {}
    if trace:
        bass_utils.upload_artifacts = lambda tmpdir: tmpdir  # no cloud store here
        kwargs = dict(trace=True, trace_cores=trace_cores or [0])
    res = bass_utils.run_bass_kernel_spmd(
        nc, in_maps, core_ids=list(range(N_CORES)), **kwargs
    )
    out = np.concatenate(
        [res.results[c]["yout"] for c in range(N_CORES)], axis=0
    ).astype(np.float32)
    return out, res


def kernel(**inputs) -> np.ndarray:
    out, _ = _run(inputs)
    return out
